# revision 1
# baseline (speedup 1.0000x reference)
"""Trainium2 Bass kernel for 2-layer LSTM (H=32, in=1) + final-step FC.

Problem: x [4096, 1024, 1] -> 2x LSTM(H=32) -> h2[:, -1, :] @ Wfc.T + bfc -> [4096, 1]

Strategy: pure data-parallel over batch (512 per core, 8 cores).
Per core, everything stays resident in SBUF; the T=1024 recurrence is fully
unrolled.  Layout is gate-major: the step matmul produces gates [4H=128
partitions, B=512 free] with weights as the stationary operand.

Per-timestep op schedule (iteration t):
  - DMA x_t row -> state slot (4-slot ring, gives the DMA ~4 steps of slack)
  - MM1: W1^T @ [x_t; h1_{t-1}]            -> G1 PSUM [128,512]
  - sigma1 = sigmoid(G1[ifo] + b1)          (ACT, bias per partition)
  - g1~    = tanh(G1[g] + b1g)
  - c1     = f1*c1 + i1*g1~                 (DVE bf16 2x)
  - th     = tanh(C[0:64])                  computes tanh(c1_t) AND tanh(c2_{t-1})
  - h1_t   = o1*th1  (written twice: rhs rows for MM1(t+1) and MM2(t))
  - h2_{t-1} = o2_{t-1}*th2                 (deferred one step; lands next to h1_t)
  - MM2: W2^T @ [h1_t; h2_{t-1}]            -> G2 PSUM [128,512]
  - sigma2 / g2~ / c2-update                (th2/h2_t deferred to iteration t+1)

Gate order is permuted from PyTorch's [i,f,g,o] to [i,f,o,g] so the three
sigmoid gates are contiguous partitions (one ACT instr) and tanh-gate separate.

The final FC ([4096,32] @ [32,1]) is done on host in numpy.
"""

import numpy as np
import ml_dtypes

BF16 = ml_dtypes.bfloat16

H = 32
T = 1024
B_TOTAL = 4096
N_CORES = 8
B = B_TOTAL // N_CORES  # 512 per core
R = 8  # x-row refill granularity (ring is 2R slots)
KERNEL_K = 1  # independent batch chains per core

_PERM = np.concatenate([
    np.arange(0, 32),      # i
    np.arange(32, 64),     # f
    np.arange(96, 128),    # o
    np.arange(64, 96),     # g
])


def build_bass(Tn=T, Bn=B, xt_rows=None, K=1, R=8, merged_tanhc=False):
    """K independent batch chains of width Bn/K; R-deep state-slot ring.

    All per-chain tiles are free-dim column slices of shared tiles, so the
    instruction structure is identical per chain and chains interleave on the
    engines to hide the per-step dependency-chain latency.

    xT input is chain-major: [K, Tn, Bc] so the once-per-R-steps x DMA for a
    chain reads a contiguous [R, Bc] block.
    """
    import concourse.bass as bass
    import concourse.bacc as bacc
    import concourse.tile as tile
    from concourse import mybir

    f32 = mybir.dt.float32
    bf16 = mybir.dt.bfloat16
    AF = mybir.ActivationFunctionType

    Bc = Bn // K
    assert Tn % R == 0

    nc = bacc.Bacc(None, target_bir_lowering=False)
    xT = nc.declare_dram_parameter("xT", [K, xt_rows or Tn, Bc], bf16, isOutput=False)
    w12 = nc.declare_dram_parameter("w12", [128, 128], bf16, isOutput=False)
    w2x = nc.declare_dram_parameter("w2x", [128, 128], bf16, isOutput=False)
    bias = nc.declare_dram_parameter("bias", [128, 2], f32, isOutput=False)
    out = nc.declare_dram_parameter("h2_last", [32, Bn], bf16, isOutput=True)

    with tile.TileContext(nc) as tc:
        with (
            tc.tile_pool(name="singles", bufs=1) as singles,
            tc.tile_pool(name="psum", bufs=8, space="PSUM") as psum,
        ):
            WS = singles.tile([128, 128], bf16)  # rows 0:33 = [Whh0;Wih0], 64:96 = Whh1
            W2X = singles.tile([128, 128], bf16)  # rows 64:128 = [Wih1; Whh1]
            BIAS = singles.tile([128, 2], f32)
            nc.sync.dma_start(WS[:], w12[:])
            nc.sync.dma_start(W2X[:], w2x[:])
            nc.sync.dma_start(BIAS[:], bias[:])

            # Big state tile; 2R slots per chain (x rows double-buffered in
            # halves of R).  rows: 0:32 h1, 32 x_t, 64:96 h2
            NS = 2 * R
            STB = singles.tile([128, K * NS * Bc], bf16)
            C = singles.tile([64, K * 2 * Bc], bf16)    # rows 32:64; L1/L2 per chain
            TH = singles.tile([96, K * 2 * Bc], bf16)   # rows 64:96
            SIG = singles.tile([96, K * 2 * Bc], bf16)  # [i;f;o]
            GT = singles.tile([32, K * 2 * Bc], bf16)
            TI = singles.tile([64, K * 2 * Bc], bf16)   # rows 32:64
            OUTT = singles.tile([32, Bn], bf16)

            def slot(c, r):
                off = (c * NS + (r % NS)) * Bc
                return STB[:, off:off + Bc]

            def lcol(tile_, c, l):  # per-(chain, layer) column slice
                off = (c * 2 + l) * Bc
                return tile_[:, off:off + Bc]

            for c in range(K):
                nc.vector.memset(slot(c, 0)[0:32, :], 0.0)      # h1_{-1}
                nc.vector.memset(slot(c, 1)[96:128, :], 0.0)    # h2_{-1}
            nc.vector.memset(C[32:64, :], 0.0)

            b1s = BIAS[0:96, 0:1]
            b1g = BIAS[96:128, 0:1]
            b2s = BIAS[0:96, 1:2]
            b2g = BIAS[96:128, 1:2]

            def xdma(c, t0):
                # rows t0..t0+R-1 of chain c -> x rows (p32) of slots t0%NS..+R-1
                s = (c * NS + (t0 % NS)) * Bc
                dst = STB[32:33, s:s + R * Bc]
                nc.sync.dma_start(dst, xT[c, t0:t0 + R, :].rearrange("t b -> (t b)")[None, :])

            for c in range(K):
                xdma(c, 0)

            for t in range(Tn):
                for c in range(K):
                    s0 = slot(c, t)
                    s1 = slot(c, t + 1)
                    sg = lcol(SIG, c, 0)
                    gt = lcol(GT, c, 0)
                    ti = lcol(TI, c, 0)
                    cc = lcol(C, c, 0)
                    th = lcol(TH, c, 0)
                    sg2 = lcol(SIG, c, 1)
                    gt2 = lcol(GT, c, 1)
                    ti2 = lcol(TI, c, 1)
                    cc2 = lcol(C, c, 1)
                    th2 = lcol(TH, c, 1)

                    G1 = psum.tile([128, Bc], f32, tag="G")
                    nc.tensor.matmul(G1[:], WS[0:33, :], s0[0:33, :],
                                     start=True, stop=True)
                    # L1 gate activations for step t
                    nc.scalar.activation(sg, G1[0:96, :], AF.Sigmoid, bias=b1s)
                    nc.scalar.activation(gt, G1[96:128, :], AF.Tanh, bias=b1g)
                    # L2 gate activations for step t-1 (G2 from last iteration)
                    if t > 0:
                        nc.scalar.activation(sg2, G2[0:96, :], AF.Sigmoid, bias=b2s)
                        nc.scalar.activation(gt2, G2[96:128, :], AF.Tanh, bias=b2g)
                    # L1 cell update (t)
                    nc.vector.tensor_mul(ti[32:64, :], sg[0:32, :], gt[0:32, :])
                    nc.vector.tensor_mul(cc[32:64, :], sg[32:64, :], cc[32:64, :])
                    nc.vector.tensor_add(cc[32:64, :], cc[32:64, :], ti[32:64, :])
                    # L2 cell update (t-1)
                    if t > 0:
                        nc.vector.tensor_mul(ti2[32:64, :], sg2[0:32, :], gt2[0:32, :])
                        nc.vector.tensor_mul(cc2[32:64, :], sg2[32:64, :], cc2[32:64, :])
                        nc.vector.tensor_add(cc2[32:64, :], cc2[32:64, :], ti2[32:64, :])
                    nc.scalar.activation(th[64:96, :], cc[32:64, :], AF.Tanh)
                    if t > 0:
                        nc.scalar.activation(th2[64:96, :], cc2[32:64, :], AF.Tanh)
                    # h1_t (both copies), h2_{t-1}
                    nc.vector.tensor_mul(s1[64:96, :], sg[64:96, :], th[64:96, :])
                    nc.vector.tensor_mul(s1[0:32, :], sg[64:96, :], th[64:96, :])
                    if t > 0:
                        nc.vector.tensor_mul(s1[96:128, :], sg2[64:96, :],
                                             th2[64:96, :])

                    G2 = psum.tile([128, Bc], f32, tag="G")
                    nc.tensor.matmul(G2[:], W2X[64:128, :], s1[64:128, :],
                                     start=True, stop=True)

                    # refill x rows for the slot ring, one DMA per R steps
                    if t % R == 0 and t + R < Tn:
                        xdma(c, t + R)

            # epilogue: finish L2 ladder for step Tn-1 and emit h2_last
            for c in range(K):
                sg2 = lcol(SIG, c, 1)
                gt2 = lcol(GT, c, 1)
                ti2 = lcol(TI, c, 1)
                cc2 = lcol(C, c, 1)
                th2 = lcol(TH, c, 1)
                nc.scalar.activation(sg2, G2[0:96, :], AF.Sigmoid, bias=b2s)
                nc.scalar.activation(gt2, G2[96:128, :], AF.Tanh, bias=b2g)
                nc.vector.tensor_mul(ti2[32:64, :], sg2[0:32, :], gt2[0:32, :])
                nc.vector.tensor_mul(cc2[32:64, :], sg2[32:64, :], cc2[32:64, :])
                nc.vector.tensor_add(cc2[32:64, :], cc2[32:64, :], ti2[32:64, :])
                nc.scalar.activation(th2[64:96, :], cc2[32:64, :], AF.Tanh)
                nc.vector.tensor_mul(OUTT[:, c * Bc:(c + 1) * Bc],
                                     sg2[64:96, :], th2[64:96, :])
            nc.sync.dma_start(out[:], OUTT[:])

    if not nc.is_finalized():
        nc.finalize()
    return nc


def _prep_shared(Wih0, Whh0, bih0, bhh0, Wih1, Whh1, bih1, bhh1):
    p = _PERM
    w12 = np.zeros((128, 128), np.float32)
    w12[0:32] = Whh0[p, :].T
    w12[32:33] = Wih0[p, 0:1].T
    w12[64:96] = Whh1[p, :].T
    w2x = np.zeros((128, 128), np.float32)
    w2x[64:96] = Wih1[p, :].T
    w2x[96:128] = Whh1[p, :].T
    bias = np.stack([(bih0 + bhh0)[p], (bih1 + bhh1)[p]], axis=1)  # [128, 2]
    return w12.astype(BF16), w2x.astype(BF16), bias.astype(np.float32)


def kernel(x, Wih0, Whh0, bih0, bhh0, Wih1, Whh1, bih1, bhh1, Wfc, bfc):
    from concourse.bass_utils import run_bass_kernel_spmd

    x = np.asarray(x, np.float32)
    w12, w2x, bias = _prep_shared(
        np.asarray(Wih0, np.float32), np.asarray(Whh0, np.float32),
        np.asarray(bih0, np.float32), np.asarray(bhh0, np.float32),
        np.asarray(Wih1, np.float32), np.asarray(Whh1, np.float32),
        np.asarray(bih1, np.float32), np.asarray(bhh1, np.float32))

    nc = build_bass(T, B, K=KERNEL_K)

    in_maps = []
    K = KERNEL_K
    Bc = B // K
    for c in range(N_CORES):
        xc = x[c * B:(c + 1) * B, :, 0]          # [B, T]
        xTc = np.stack([np.ascontiguousarray(xc[k * Bc:(k + 1) * Bc, :].T)
                        for k in range(K)], axis=0).astype(BF16)  # [K, T, Bc]
        in_maps.append({"xT": xTc, "w12": w12, "w2x": w2x, "bias": bias})

    res = run_bass_kernel_spmd(nc, in_maps, core_ids=list(range(N_CORES)))

    Wfc = np.asarray(Wfc, np.float32)
    bfc = np.asarray(bfc, np.float32)
    outs = []
    for c in range(N_CORES):
        h2 = np.asarray(res.results[c]["h2_last"], dtype=np.float32)  # [32, B]
        outs.append(h2.T @ Wfc.T + bfc)          # [B, 1]
    return np.concatenate(outs, axis=0).astype(np.float32)

